# revision 81
# baseline (speedup 1.0000x reference)
"""BitConv2d (ternary-quantized 3x3 conv) on 8 Trainium2 NeuronCores.

Contract: kernel(**inputs) takes FULL unsharded inputs
  x [32, 256, 56, 56] f32, weight [256, 256, 3, 3] f32, bias [256] f32,
  scale_ema scalar f32
and returns the FULL output y [32, 256, 56, 56] f32.

Strategy: data-parallel over batch (4 images / core), weights replicated.
  Pass 1 (device): per-core max(|x_shard|) -> host combine -> beta.
  Host: quantize weights (tiny: 590K elems, bit-exact f32 replication of the
        reference formula), cast to fp8 e4m3, fold scalars.
  Pass 2 (device): quantize x to integer-valued f16 (exact), split each
        integer u exactly into u = hi + lo with both halves representable
        in fp8 e4m3 (hi = e4m3-rounding of u, lo = integer residual <= 4).
        3x3 conv via fp8 DoubleRow matmuls: each instruction contracts
        K=256 (both cin chunks) at 0.5 cycles/output-column, and the hi/lo
        halves accumulate into the same PSUM tile with identical weights,
        so the conv runs at 2x the fp16 matmul rate. Epilogue
        beta*gamma*acc + bias, write f32.
"""

import numpy as np
import ml_dtypes

import concourse.bass as bass
import concourse.tile as tile
from concourse import bacc, mybir
from concourse.bass_interp import get_hw_module
from concourse.bass_utils import run_bass_kernel_spmd

_NCORES = 8
_MAGIC16 = 1536.0   # 1.5 * 2**10: f16 ulp is 1 -> forces round-to-nearest-int
_MAGIC8 = 12288.0   # 1.5 * 2**13: f16 ulp is 8 -> rounds to a multiple of 8
_F32 = mybir.dt.float32
_F16 = mybir.dt.float16
_F8 = mybir.dt.float8e4
_DR = mybir.MatmulPerfMode.DoubleRow

# results of the last kernel() call, for test.py introspection
last_results = {}


def _build_max_kernel(nsh, cin, h, w):
    """Per-core abs-max over the x shard -> mx [128,1] (partition partials)."""
    nc = bacc.Bacc("TRN2", target_bir_lowering=False, debug=False,
                   num_devices=_NCORES)
    x = nc.dram_tensor("x", [nsh, cin, h, w], _F32, kind="ExternalInput")
    cinc = cin // 128
    # quarter-chunk granularity so the final reduce tail is short; the very
    # last plane tapers geometrically so the post-DMA reduce chain is tiny
    hw4 = (h * w) // 4
    base = [hw4] * 4
    taper = [hw4 // 2] * 7 + [hw4 // 4, hw4 // 4]   # all chunks >= 512B
    assert sum(taper) == h * w
    ntiles = (nsh * cinc - 1) * len(base) + len(taper)
    mx = nc.dram_tensor("mx", [128, 1], _F32, kind="ExternalOutput")
    with tile.TileContext(nc, trace_sim=False) as tc:
        with tc.tile_pool(name="xs", bufs=4) as xs, \
             tc.tile_pool(name="acc", bufs=1) as accp:
            pm = accp.tile([128, ntiles], _F32)
            k = 0
            for n in range(nsh):
                for c in range(cinc):
                    last = (n == nsh - 1 and c == cinc - 1)
                    chunks = taper if last else base
                    xt = xs.tile([128, h * w], _F32, name="xt", tag="xt")
                    q0 = 0
                    for qlen in chunks:
                        sl = xt[:, q0:q0 + qlen]
                        nc.sync.dma_start(
                            sl, x.ap()[n, c * 128:(c + 1) * 128]
                            .rearrange("p a b -> p (a b)")
                            [:, q0:q0 + qlen])
                        nc.vector.reduce_max(pm[:, k:k + 1], sl,
                                             axis=mybir.AxisListType.X,
                                             apply_absolute_value=True)
                        k += 1
                        q0 += qlen
            mxt = accp.tile([128, 1], _F32)
            nc.vector.reduce_max(mxt[:], pm[:], axis=mybir.AxisListType.X)
            nc.sync.dma_start(mx.ap(), mxt[:])
    nc.compile()
    nc.m = get_hw_module(nc.m)
    return nc


def _build_conv_kernel(nsh, cin, cout, h, w):
    """Quantize x + 3x3 same-pad conv with prequantized fp8 weights.

    Inputs per core:
      x  [nsh, cin, h, w] f32
      wq [128, 9, cin//128, cout] fp8e4  (ci_low-major lhsT layout)
      b  [cout//128, 128, 1] f32
      sc [128, 2] f32                    (inv_beta, beta*gamma) broadcast rows
    Output: y [nsh, cout, h, w] f32
    """
    assert h == w == 56 and cin % 128 == 0 and cout % 128 == 0
    cinc, coc = cin // 128, cout // 128
    hp, wp = h + 2, w + 2          # padded plane 58x58
    NR = 4                         # output rows per PE tile
    NW = (NR - 1) * wp + w         # flattened moving window = 230 columns
    rowg = h // NR                 # 14 tiles per image
    Ident = mybir.ActivationFunctionType.Identity
    Sub = mybir.AluOpType.subtract

    nc = bacc.Bacc("TRN2", target_bir_lowering=False, debug=False,
                   num_devices=_NCORES)
    x = nc.dram_tensor("x", [nsh, cin, h, w], _F32, kind="ExternalInput")
    wq = nc.dram_tensor("wq", [128, 9, cinc, cout], _F8, kind="ExternalInput")
    b = nc.dram_tensor("b", [coc, 128, 1], _F32, kind="ExternalInput")
    sc = nc.dram_tensor("sc", [128, 2], _F32, kind="ExternalInput")
    y = nc.dram_tensor("y", [nsh, cout, h, w], _F32, kind="ExternalOutput")

    with tile.TileContext(nc, trace_sim=False) as tc:
        with tc.tile_pool(name="const", bufs=1) as const, \
             tc.tile_pool(name="xstage", bufs=4) as xstage, \
             tc.tile_pool(name="qstage", bufs=4) as qstage, \
             tc.tile_pool(name="outs", bufs=18) as outs, \
             tc.tile_pool(name="psum", bufs=8, space="PSUM") as psum:

            # ---- constants -------------------------------------------------
            # preload the ACT function table (lazy-load costs 1.3us on the
            # first activation otherwise)
            scratch = const.tile([128, 1], _F32)
            nc.scalar.activation(scratch[:],
                                 nc.const_aps.tensor(0.0, (128, 1)), Ident)
            # warm the PE while the head DMAs run: dummy matmuls on zeros
            # keep the activity window busy so the first real matmuls run
            # at 2.4GHz instead of the cold 1.2GHz
            zw = const.tile([128, 128], _F16)
            nc.gpsimd.memset(zw[:], 0.0)   # Pool starts ~0.7us before DVE
            psw = psum.tile([128, 128], _F32, name="psw", tag="ps")
            for _ in range(68):
                nc.tensor.matmul(psw[:], zw[:], zw[:], start=True, stop=True)

            w_sb = const.tile([128, 9, cinc, cout], _F8)
            sc_sb = const.tile([128, 2], _F32)
            b_sb = const.tile([128, coc], _F32)
            mg_p = const.tile([128, 1], _F32)
            nc.vector.memset(mg_p[:], _MAGIC16)
            mg_n = const.tile([128, 1], _F32)
            nc.vector.memset(mg_n[:], -_MAGIC16)

            def _load_consts():
                # sc gates the whole quantize chain: lead the HWDGE queue
                # with it (tiny). Bulk weights + bias go via SWDGE, emitted
                # later (after the first two x chunks) so the 1.6us weight
                # transfer doesn't hog the DMA bus ahead of the x chunks
                # that gate the first quantize ops
                nc.sync.dma_start(sc_sb[:], sc.ap())

            def _load_weights():
                # on the sync queue, emitted after the first two x chunks:
                # DMA-bus arbitration is by ready time, and a SWDGE-triggered
                # weight DMA would otherwise slip ahead of the x chunks that
                # gate the whole quantize chain. Two halves so each bus slot
                # stays short. bias rides the idle SWDGE queue.
                nc.sync.dma_start(
                    w_sb[:, 0:5].rearrange("p t c f -> p (t c f)"),
                    wq.ap()[:, 0:5].rearrange("p t c f -> p (t c f)"))
                nc.sync.dma_start(
                    w_sb[:, 5:9].rearrange("p t c f -> p (t c f)"),
                    wq.ap()[:, 5:9].rearrange("p t c f -> p (t c f)"))
                nc.gpsimd.dma_start(b_sb[:],
                                    b.ap().rearrange("c p o -> p (c o)"))

            # sc is emitted inside the first quant unit, right after its x
            # DMA: the long x transfer leads the bus, the tiny sc rides
            # directly behind it

            # ---- padded quantized input planes (fp8, zero borders) ---------
            # layout [128, hl, cinc, img, 58, 58]: hl=0 is the e4m3 rounding
            # of the integer activation u, hl=1 the exact residual u - hi
            xq = const.tile([128, 2, cinc, nsh, hp, wp], _F8)
            for s in range(2):
                for c in range(cinc):
                    nc.gpsimd.memset(xq[:, s, c, :, 0, :], 0.0)
                    nc.gpsimd.memset(xq[:, s, c, :, hp - 1, :], 0.0)
                    nc.gpsimd.memset(xq[:, s, c, :, :, 0], 0.0)
                    nc.gpsimd.memset(xq[:, s, c, :, :, wp - 1], 0.0)

            # u = round_half_even(x * inv_beta); |x*inv_beta| < 127 by
            # construction so no clip is needed. hi = e4m3(u) (engine write
            # cast rounds to nearest), lo = u - hi is an integer <= 4 in
            # magnitude, exact in e4m3, so hi + lo == u exactly.
            state = {"qi": 0}

            def _quant_unit(n, r, rch, c):
                xt = xstage.tile([128, rch, w], _F32, name="xt", tag="xt")
                nc.sync.dma_start(
                    xt[:], x.ap()[n, c * 128:(c + 1) * 128, r:r + rch, :])
                if state["qi"] == 0:
                    _load_consts()
                t16 = qstage.tile([128, rch, w], _F16, name="t16", tag="t16")
                # pass 1: u16 = x*inv_beta + 1536 (f16 write rounds the sum
                # to an exact integer + 1536). Unit 1 runs its whole chain
                # on DVE so the two head units pipeline on separate engines.
                if state["qi"] == 1:
                    nc.vector.tensor_scalar(t16[:], xt[:], sc_sb[:, 0:1],
                                            _MAGIC16,
                                            op0=mybir.AluOpType.mult,
                                            op1=mybir.AluOpType.add)
                else:
                    nc.scalar.activation(t16[:], xt[:], Ident, bias=mg_p[:],
                                         scale=sc_sb[:, 0:1])
                hi_v = xq[:, 0, c, n, 1 + r:1 + r + rch, 1:w + 1]
                lo_v = xq[:, 1, c, n, 1 + r:1 + r + rch, 1:w + 1]
                # pass 2: hi = (t16 - 1536) cast to e4m3. During image 0 the
                # PE is chasing fresh rows while epilogues already compete
                # for ACT/DVE, so image 0 alternates hi onto the idle Pool
                # engine (slow per-element but off the critical lanes).
                if state["qi"] % 2 == 0:
                    nc.scalar.activation(hi_v, t16[:], Ident, bias=mg_n[:])
                elif n == 0 and state["qi"] >= 2:
                    nc.gpsimd.tensor_scalar(hi_v, t16[:], -_MAGIC16, None,
                                            op0=mybir.AluOpType.add)
                else:
                    nc.vector.tensor_scalar(hi_v, t16[:], -_MAGIC16, None,
                                            op0=mybir.AluOpType.add)
                # pass 3: lo = (t16 - 1536) - hi, fused (DVE only:
                # scalar_tensor_tensor fails the Pool ISA check)
                nc.vector.scalar_tensor_tensor(lo_v, t16[:], _MAGIC16, hi_v,
                                               op0=Sub, op1=Sub)
                state["qi"] += 1

            def _quant_units(n):
                # row-chunks x 2 cin chunks, c fastest so the first rows of
                # both chunks (which gate the first matmul) arrive first.
                # image 0 is chunked fine so the PE pipeline starts early
                rch = h // (8 if n == 0 else 4)
                return [(n, r, rch, c)
                        for r in range(0, h, rch) for c in range(cinc)]

            # ---- conv: 18 DoubleRow matmuls per [128co x 4h x 56w] tile ----
            # rhs = flattened 230-column window of the padded plane (both
            # cin chunks stacked in the DoubleRow pair dim); columns at the
            # row wrap positions compute garbage and are skipped on readout
            # epilogues of two adjacent row-tiles share one out buffer and
            # one merged y DMA (halves the HWDGE instruction load, which
            # otherwise delays the just-in-time x loads behind 112 y writes)
            state["ot"] = None
            state["defer"] = []

            def _mm_group(n, ps, h0, nr, co):
                # moving operand as a 4-dim strided AP [128, 2(cinc),
                # nr(row stride 58), 56]: only valid columns stream through
                # the PE — no junk columns from flattened windows
                for s in range(2):
                    for tap in range(9):
                        dh, dw = tap // 3, tap % 3
                        nc.tensor.matmul(
                            ps[:],
                            w_sb[:, tap, :, co * 128:(co + 1) * 128],
                            xq[:, s, :, n, h0 + dh:h0 + dh + nr,
                               dw:dw + w],
                            start=(s == 0 and tap == 0),
                            stop=(s == 1 and tap == 8),
                            perf_mode=_DR)

            def _epi_compute(dst, ps, nr, co, on_dve):
                pv = ps[:]
                if on_dve:
                    nc.vector.tensor_scalar(dst, pv, sc_sb[:, 1:2],
                                            b_sb[:, co:co + 1],
                                            op0=mybir.AluOpType.mult,
                                            op1=mybir.AluOpType.add)
                else:
                    nc.scalar.activation(dst, pv, Ident,
                                         bias=b_sb[:, co:co + 1],
                                         scale=sc_sb[:, 1:2])

            def _epilogue(ps, idx, n, h0, co):
                if state["ot"][co] is None:
                    state["ot"][co] = outs.tile([128, 2, NR, w], _F32,
                                                name="ot", tag="ot")
                ot = state["ot"][co]
                half = (h0 // NR) % 2
                _epi_compute(ot[:, half], ps, NR, co, idx % 2 == 0)
                if half == 1:
                    dst = y.ap()[n, co * 128:(co + 1) * 128,
                                 h0 - NR:h0 + NR, :]
                    src = ot[:].rearrange("p a r q -> p (a r) q")
                    if n == 0:
                        # image 0's writebacks are deferred: the DMA bus is
                        # the binding resource while the x feed fills the
                        # pipeline, so these transfers trickle out during
                        # image 1's tiles instead
                        state["defer"].append((dst, src))
                    else:
                        nc.sync.dma_start(dst, src)
                    state["ot"][co] = None

            # emission order defines each engine's execution order: image
            # n+1's quantize units are spread between image n's matmul
            # tiles so epilogues never queue behind a full image of
            # quantize work on ACT/DVE (which would stall the PE once all
            # PSUM bufs are pending drain). co is the fastest tile axis so
            # each quantized row range feeds two tiles (halves the rate at
            # which the PE chases fresh rows during image 0).
            idx = 0
            for ui, u in enumerate(_quant_units(0)):
                _quant_unit(*u)
                if ui == 1:
                    _load_weights()
            for n in range(nsh):
                pending = _quant_units(n + 1) if n + 1 < nsh else []
                state["ot"] = [None] * coc
                if n == nsh - 1:
                    # last image: its data is already quantized, so run
                    # co-major — all but the final tiles' writebacks retire
                    # long before the tail
                    tiles = [(co, ti) for co in range(coc)
                             for ti in range(rowg)][:-1]
                else:
                    tiles = [(co, ti) for ti in range(rowg)
                             for co in range(coc)]
                for j, (co, ti) in enumerate(tiles):
                    h0 = NR * ti
                    ps = psum.tile([128, NR, w], _F32, name="ps", tag="ps")
                    _mm_group(n, ps, h0, NR, co)
                    if n == nsh - 1 and co == coc - 1 and ti == rowg - 2:
                        # the final co's penultimate tile writes back
                        # unmerged so the tail carries only the split tile
                        ot = outs.tile([128, NR, w], _F32, name="ot",
                                       tag="ot")
                        _epi_compute(ot[:], ps, NR, co, idx % 2 == 0)
                        nc.sync.dma_start(
                            y.ap()[n, co * 128:(co + 1) * 128,
                                   h0:h0 + NR, :], ot[:])
                    else:
                        _epilogue(ps, idx, n, h0, co)
                    idx += 1
                    if pending and j % 3 == 2:
                        _quant_unit(*pending.pop(0))
                    if state["defer"] and n >= 1 and j % 2 == 0:
                        nc.sync.dma_start(*state["defer"].pop(0))
                for u in pending:
                    _quant_unit(*u)
                if n == nsh - 1:
                    # very last tile: a 3-row piece then a 1-row piece with
                    # epilogues on both engines and one shared writeback, so
                    # the post-matmul tail chain is as short as possible
                    co, h0 = coc - 1, NR * (rowg - 1)
                    ot = outs.tile([128, NR, w], _F32, name="ot", tag="ot")
                    for piece, (hh, nr) in enumerate([(h0, 3), (h0 + 3, 1)]):
                        ps = psum.tile([128, nr, w], _F32, name="ps",
                                       tag="ps")
                        _mm_group(n, ps, hh, nr, co)
                        _epi_compute(ot[:, hh - h0:hh - h0 + nr], ps, nr,
                                     co, piece == 0)
                    nc.sync.dma_start(
                        y.ap()[n, co * 128:(co + 1) * 128,
                               h0:h0 + NR, :], ot[:])
    nc.compile()
    nc.m = get_hw_module(nc.m)
    return nc


_cache = {}


def _get(builder, *args):
    key = (builder.__name__,) + args
    if key not in _cache:
        _cache[key] = builder(*args)
    return _cache[key]


def _run(nc, in_maps, cores):
    """run_bass_kernel_spmd with retries for transient device errors."""
    import time
    last = None
    for attempt in range(3):
        try:
            return run_bass_kernel_spmd(nc, in_maps, cores)
        except Exception as e:
            last = e
            time.sleep(2.0 * (attempt + 1))
    raise last


def _quantize_weights(weight, gamma):
    """Bit-exact f32 replication of the reference chimera-ternary transform."""
    f32 = np.float32
    ws = (weight / gamma).astype(f32)
    tern = np.clip(np.round(ws), f32(-1.0), f32(1.0)).astype(f32)
    raw = (f32(1.0 - 0.7) * ws + f32(0.7) * tern).astype(f32)
    # straight-through estimator is an fp identity only up to rounding:
    # replicate w + (raw - w) op-for-op, then clamp
    ste = (weight + (raw - weight)).astype(f32)
    return np.clip(ste, f32(-1.0), f32(1.0)).astype(f32)


def kernel(x, weight, bias, scale_ema):
    x = np.ascontiguousarray(x, dtype=np.float32)
    weight = np.ascontiguousarray(weight, dtype=np.float32)
    bias = np.ascontiguousarray(bias, dtype=np.float32)
    f32 = np.float32
    N, cin, h, w = x.shape
    cout = weight.shape[0]
    nsh = N // _NCORES
    cores = list(range(_NCORES))

    # ---- host-side tiny prep (beta-independent, done before launch 1 so
    # the gap between the two device launches is only scalar math) ---------
    gamma = np.maximum(f32(scale_ema), f32(1e-6))
    wqf = _quantize_weights(weight, gamma)
    # [cout, cin, 3, 3] -> [ci_low(128), tap, ci_chunk, co]  (lhsT layout)
    wql = np.ascontiguousarray(
        wqf.transpose(1, 2, 3, 0)                  # [cin, 3, 3, cout]
        .reshape(cin // 128, 128, 9, cout)
        .transpose(1, 2, 0, 3)                     # [128, 9, cinc, cout]
    ).astype(ml_dtypes.float8_e4m3)
    b_l = np.ascontiguousarray(bias.reshape(cout // 128, 128, 1))
    ncA = _get(_build_max_kernel, nsh, cin, h, w)
    ncB = _get(_build_conv_kernel, nsh, cin, cout, h, w)

    # ---- pass 1: global abs-max -> beta ---------------------------------
    resA = _run(ncA, [{"x": x[i * nsh:(i + 1) * nsh]} for i in cores], cores)
    last_results["max"] = resA
    gmax = f32(max(f32(r["mx"].max()) for r in resA.results))
    beta = gmax / f32(127.0) + f32(1e-6)
    sc = np.tile(np.array([f32(1.0) / beta, beta * gamma], f32), (128, 1))
    sc = np.ascontiguousarray(sc)

    # ---- pass 2: quantize x + conv --------------------------------------
    in_maps = [{"x": x[i * nsh:(i + 1) * nsh], "wq": wql, "b": b_l, "sc": sc}
               for i in cores]
    resB = _run(ncB, in_maps, cores)
    last_results["conv"] = resB
    return np.concatenate([resB.results[i]["y"] for i in cores], axis=0)
